# revision 29
# baseline (speedup 1.0000x reference)
"""Trainium2 Bass kernel for per-head 3-layer MLP + softmax (nn_Clip).

Reference computation (per head h of 16, batch B=32768):
    h1 = relu(emb @ W1[h] + b1[h])          [B, 128]
    h2 = relu(h1 @ W2[h] + b2[h])           [B, 64]
    out[h] = softmax(h2 @ W3[h] + b3[h])    [B, 10]

Strategy: data-parallel over batch across 8 NeuronCores (4096 rows each),
per-head MLP weights replicated. Layers 1-2 run in fp8e4 with DoubleRow
(SwInterleave) at the 157 TF/s fp8 peak; layer 3 in bf16.

Layout (per core):
  - emb shipped pre-transposed fp8 as embT [768, 4096]; layer-1 rhs tiles
    [e_chunk=128, b=512] load contiguously.
  - Layer 1 (feature-major): psum[d1=128, b=512] += via 3 DR matmuls of
    2 interleaved 128-chunks each (moving operand at its 1024-elem max).
  - Layer 2: heads paired; head 2j -> psum partitions 0:64, head 2j+1 ->
    64:128 via a block-diagonal DR stationary; one relu per head pair.
  - Layer 3 (batch-major): per pair j one matmul per 128-row batch slice
    writing ONLY that pair's 20 output columns (stationary = h2 slice,
    moving = dense [128, 20] W3 block); bias b3 seeds the psum via a
    rank-1 ones @ [b3|b3] matmul (start=True) per 512-col psum half.
  - PE stream is software-pipelined: L1(pair P) || L2(P-1) || L3(P-2), so
    the relu latency (ScalarE/VectorE) never stalls the Tensor engine.
  - Softmax on halves [128, 2*160]: exp on ScalarE, grouped reduce/
    reciprocal on VectorE, broadcast-multiply + store DMA on GpSimd.
"""

import numpy as np
import ml_dtypes
from contextlib import ExitStack

from concourse import bacc, bass, mybir, tile
from concourse.bass_utils import run_bass_kernel_spmd

N_CORES = 8
B = 32768
H = 16
E = 768
D1 = 128
D2 = 64
C = 10
B_LOC = B // N_CORES      # 4096 rows per core
B_TILE = 512              # batch tile (matmul free dim)
N_BT = B_LOC // B_TILE    # 8 tiles per core
KC = E // 128             # 6 contraction chunks for layer 1
NPAIR = H // 2            # 8 head pairs
OUTC = H * C              # 160 output columns per row
C2 = 2 * C                # 20 output columns per head pair
N_SUB = B_TILE // 128     # 4 batch sub-tiles of 128 for layer 3

BF16 = mybir.dt.bfloat16
F8 = mybir.dt.float8e4
F32 = mybir.dt.float32
AF = mybir.ActivationFunctionType
ALU = mybir.AluOpType
DRSW = mybir.MatmulPerfMode.DoubleRowSwInterleave

_bf = ml_dtypes.bfloat16
_f8 = ml_dtypes.float8_e4m3


def build_program(reps=1, variant="full"):
    nc = bacc.Bacc("TRN2", target_bir_lowering=False, debug=False,
                   num_devices=N_CORES)
    embT = nc.dram_tensor("embT", [E, B_LOC], F8, kind="ExternalInput").ap()
    w1p = nc.dram_tensor("w1p", [128, H * KC * 128], F8, kind="ExternalInput").ap()
    w2p = nc.dram_tensor("w2p", [128, NPAIR * 256], F8, kind="ExternalInput").ap()
    w3p = nc.dram_tensor("w3p", [128, NPAIR * C2], BF16, kind="ExternalInput").ap()
    b1p = nc.dram_tensor("b1p", [128, H], F32, kind="ExternalInput").ap()
    b2p = nc.dram_tensor("b2p", [128, NPAIR], F32, kind="ExternalInput").ap()
    b3p = nc.dram_tensor("b3p", [1, 2 * OUTC], BF16, kind="ExternalInput").ap()
    out = nc.dram_tensor("out", [B_LOC, OUTC], F32, kind="ExternalOutput").ap()

    with tile.TileContext(nc) as tc:
        with ExitStack() as ctx:
            # PSUM pools are shared across reps (only 8 banks exist); SBUF
            # pools alternate between two live sets so rep k+1's DMA loads
            # overlap rep k's compute instead of waiting on address reuse.
            # "pp": shift one bank from ps2 to ps3 so the exp drain of tile
            # bt doesn't gate tile bt+1's bias matmul.
            pp = "pp0" not in variant
            pq = "pq" in variant
            psum = dict(
                ps1=ctx.enter_context(
                    tc.tile_pool(name="ps1", bufs=3 if pq else 4,
                                 space="PSUM")),
                ps2=ctx.enter_context(
                    tc.tile_pool(name="ps2", bufs=1 if (pp or pq) else 2,
                                 space="PSUM")),
                ps3=ctx.enter_context(
                    tc.tile_pool(name="ps3",
                                 bufs=4 if pq else (3 if pp else 2),
                                 space="PSUM")),
            )
            nsets = 2 if reps > 1 else 1
            sets = []
            for s in range(nsets):
                sets.append(dict(
                    const=ctx.enter_context(
                        tc.tile_pool(name=f"const{s}", bufs=1)),
                    embp=ctx.enter_context(
                        tc.tile_pool(name=f"embp{s}", bufs=4)),
                    h1pool=ctx.enter_context(
                        tc.tile_pool(name=f"h1pool{s}", bufs=8)),
                    h2pool=ctx.enter_context(
                        tc.tile_pool(name=f"h2pool{s}", bufs=16)),
                    smp=ctx.enter_context(
                        tc.tile_pool(name=f"smp{s}", bufs=2)),
                ))
            for r in range(reps):
                _body(tc, {**psum, **sets[r % nsets]},
                      embT, w1p, w2p, w3p, b1p, b2p, b3p, out,
                      variant=variant)
    nc.compile()
    return nc


def _body(tc, pools, embT, w1p, w2p, w3p, b1p, b2p, b3p, out, variant="full"):
    nc = tc.nc
    base, _, flags = variant.partition("+")
    do_act = base in ("full", "nosm")
    do_sm = base in ("full",)
    do_l2 = base not in ("l1",)
    do_l3 = base not in ("l1", "l12")
    # h2 dtype: fp8 halves the layer-3 LDWEIGHTS stream if the weight-plane
    # load rate is byte-based; bf16 otherwise.
    h2_dt = F8 if "h28" in flags else BF16
    # emb tiles load on the ScalarE DGE queue, weights on the SP queue:
    # separate queues let the per-tile emb prefetch bypass the bulk weight
    # transfers at each rep boundary.
    es_eng = nc.sync if "spq" in flags else nc.scalar
    w1_eng = nc.gpsimd if "wq" in flags else nc.sync
    const = pools["const"]
    embp = pools["embp"]
    h1pool = pools["h1pool"]
    h2pool = pools["h2pool"]
    smp = pools["smp"]
    ps1 = pools["ps1"]
    ps2 = pools["ps2"]
    ps3 = pools["ps3"]

    embT3 = embT.rearrange("(k e) b -> e k b", e=128)
    # First emb tile loads before the weights on the SP queue so layer 1
    # can start as early as possible; weights follow on the same queue.
    es_tiles = {}
    es_tiles[0] = embp.tile([128, KC, B_TILE], F8, tag="emb", name="es0")
    # the first tile of each rep rides the SP queue, which is idle at the
    # rep boundary (the ScalarE queue still holds the previous rep's tail
    # relus/exps, which would delay the dma issue).
    es0_eng = es_eng if "es0sq" in flags else nc.sync
    es0_eng.dma_start(es_tiles[0][:], embT3[:, :, 0:B_TILE])
    b1_sb = const.tile([128, H], F32)
    nc.sync.dma_start(b1_sb[:], b1p[:])
    b2_sb = const.tile([128, NPAIR], F32)
    nc.sync.dma_start(b2_sb[:], b2p[:])
    b3_sb = const.tile([1, 2 * OUTC], BF16)
    nc.sync.dma_start(b3_sb[:], b3p[:])
    w1_sb = const.tile([128, H * KC, 128], F8)
    w1p3 = w1p[:].rearrange("p (t m) -> p t m", m=128)
    for j in range(NPAIR):
        t0 = 2 * j * KC
        t1 = 2 * (j + 1) * KC
        w1_eng.dma_start(w1_sb[:, t0:t1, :], w1p3[:, t0:t1, :])
    w2_sb = const.tile([128, NPAIR, 256], F8)
    nc.sync.dma_start(w2_sb[:], w2p[:].rearrange("p (j t) -> p j t", t=256))
    w3_sb = const.tile([128, NPAIR * C2], BF16)
    nc.sync.dma_start(w3_sb[:], w3p[:])
    ones_sb = const.tile([1, 128], BF16)
    nc.vector.memset(ones_sb[:], 1.0)
    if not do_act:
        h1_dummy = const.tile([128, 2, B_TILE], F8)
        nc.vector.memset(h1_dummy[:], 0.25)
        h2_dummy = const.tile([128, B_TILE], h2_dt)
        nc.vector.memset(h2_dummy[:], 0.25)
    if not do_sm:
        out_dummy = const.tile([128, 2 * OUTC], F32)
        nc.vector.memset(out_dummy[:], 0.125)

    # Pipeline state: pair index P runs over all N_BT*NPAIR pairs; stage
    # offsets keep L2 one pair and L3 two pairs behind L1 so the relus
    # (ScalarE/VectorE) complete off the Tensor engine's critical path.
    NP_ALL = N_BT * NPAIR
    lag = 2 if "lag2" in flags else 1
    h1_of = {}
    h2_of = {}
    p3_of = {}
    nrelu = 0

    def softmax_store(bt):
        halves = p3_of.pop(bt) if do_l3 else (None, None)
        for half in range(2):
            ph = halves[half]
            rsl = slice(bt * B_TILE + half * 256, bt * B_TILE + (half + 1) * 256)
            if not do_sm:
                nc.gpsimd.dma_start(
                    out[rsl, :].rearrange("(m p) c -> p m c", p=128),
                    out_dummy[:].rearrange("p (m c) -> p m c", c=OUTC),
                )
                continue
            ex = smp.tile([128, 2 * OUTC], F32, tag="ex")
            nc.scalar.activation(ex[:], ph[:, 0:2 * OUTC], AF.Exp)
            G2 = 2 * H  # 32 softmax groups of width C per partition
            sums = smp.tile([128, G2], F32, tag="sums")
            nc.vector.reduce_sum(sums[:],
                                 ex[:].rearrange("p (g c) -> p g c", c=C),
                                 axis=mybir.AxisListType.X)
            nc.vector.reciprocal(sums[:], sums[:])
            outt = smp.tile([128, 2 * OUTC], F32, tag="outt")
            nc.gpsimd.tensor_mul(
                outt[:].rearrange("p (g c) -> p g c", c=C),
                ex[:].rearrange("p (g c) -> p g c", c=C),
                sums[:][:, :, None].broadcast_to((128, G2, C)),
            )
            nc.gpsimd.dma_start(
                out[rsl, :].rearrange("(m p) c -> p m c", p=128),
                outt[:].rearrange("p (m c) -> p m c", c=OUTC),
            )

    for P in range(NP_ALL + 1 + lag):
        # --- stage A: layer-1 matmuls + relu for pair P ---
        if P < NP_ALL:
            bt, j = divmod(P, NPAIR)
            if j == 0 and bt + 1 < N_BT:
                nxt = embp.tile([128, KC, B_TILE], F8, tag="emb")
                es_eng.dma_start(
                    nxt[:],
                    embT3[:, :, (bt + 1) * B_TILE:(bt + 2) * B_TILE])
                es_tiles[bt + 1] = nxt
            es = es_tiles[bt]
            h1pair = h1pool.tile([128, 2, B_TILE], F8, tag="h1")
            for hi, h in enumerate((2 * j, 2 * j + 1)):
                p1 = ps1.tile([128, B_TILE], F32, tag="p1")
                for k in range(0, KC, 2):
                    nc.tensor.matmul(
                        p1[:],
                        w1_sb[:, h * KC + k:h * KC + k + 2, :],
                        es[:, k:k + 2, :],
                        start=(k == 0),
                        stop=(k == KC - 2),
                        perf_mode=DRSW,
                    )
                if do_act:
                    on_act = (nrelu % 2 == 0)
                    if "tl0" not in flags and j >= 6:
                        on_act = False  # keep ScalarE clear for the exp
                    if on_act:
                        nc.scalar.activation(h1pair[:, hi, :], p1[:], AF.Relu,
                                             bias=b1_sb[:, h:h + 1])
                    else:
                        nc.vector.tensor_scalar(h1pair[:, hi, :], p1[:],
                                                b1_sb[:, h:h + 1],
                                                0.0, ALU.add, ALU.max)
                    nrelu += 1
            h1_of[P] = h1pair if do_act else h1_dummy
            if not do_l3 and j == NPAIR - 1:
                softmax_store(bt)

        # --- stage B: layer-2 matmul + relu for pair P-lag ---
        Q = P - lag
        if do_l2 and 0 <= Q < NP_ALL:
            j = Q % NPAIR
            h1pair = h1_of.pop(Q)
            p2 = ps2.tile([128, B_TILE], F32, tag="p2")
            nc.tensor.matmul(p2[:],
                             w2_sb[:, j, :].rearrange("p (t m) -> p t m",
                                                      m=128),
                             h1pair[:], start=True, stop=True,
                             perf_mode=DRSW)
            if do_act:
                h2 = h2pool.tile([128, B_TILE], h2_dt, tag="h2")
                on_act = (nrelu % 2 == 0)
                if "tl0" not in flags and j >= 6:
                    on_act = False
                if on_act:
                    nc.scalar.activation(h2[:], p2[:], AF.Relu,
                                         bias=b2_sb[:, j:j + 1])
                else:
                    nc.vector.tensor_scalar(h2[:], p2[:], b2_sb[:, j:j + 1],
                                            0.0, ALU.add, ALU.max)
                nrelu += 1
            else:
                h2 = h2_dummy
            h2_of[Q] = h2

        # --- stage C: layer-3 matmuls for pair P-lag-1 ---
        Q = P - lag - 1
        if do_l3 and Q >= 0:
            bt, j = divmod(Q, NPAIR)
            if j == 0:
                halves = (ps3.tile([128, 512], F32, tag="p3", name="p3a"),
                          ps3.tile([128, 512], F32, tag="p3", name="p3b"))
                p3_of[bt] = halves
                for ph in halves:
                    nc.tensor.matmul(ph[:, 0:2 * OUTC], ones_sb[:1, :],
                                     b3_sb[:1, :], start=True, stop=False)
            halves = p3_of[bt]
            h2 = h2_of.pop(Q)
            w3blk = w3_sb[:, j * C2:(j + 1) * C2]
            for m in range(N_SUB):
                ph = halves[m // 2]
                off = (m % 2) * OUTC + j * C2
                nc.tensor.matmul(ph[:, off:off + C2],
                                 h2[:, m * 128:(m + 1) * 128],
                                 w3blk, start=False,
                                 stop=(j == NPAIR - 1 and m % 2 == 1))
            if j == NPAIR - 1:
                softmax_store(bt)


def prep_inputs(clip_embedding, W1, b1, W2, b2, W3, b3):
    """Host-side prepack: cast/transpose into the layouts the kernel DMAs."""
    emb = np.asarray(clip_embedding, dtype=np.float32)
    W1 = np.asarray(W1, dtype=np.float32)
    b1 = np.asarray(b1, dtype=np.float32)
    W2 = np.asarray(W2, dtype=np.float32)
    b2 = np.asarray(b2, dtype=np.float32)
    W3 = np.asarray(W3, dtype=np.float32)
    b3 = np.asarray(b3, dtype=np.float32)

    embT = np.ascontiguousarray(emb.astype(_f8).T)              # [768, B]
    # SwInterleave layout per chunk pair (A=chunk k, B=chunk k+1), stored
    # column order [A127, B127, A126, B126, ..., A0, B0] (see bass_interp).
    w1c = W1.astype(np.float32).reshape(H, KC, 128, D1)          # [h,k,e,d]
    w1p = np.zeros((128, H * KC * D1), dtype=np.float32)
    for h in range(H):
        for kp in range(KC // 2):
            A = w1c[h, 2 * kp]       # [e,d] weights for even chunk
            Bm = w1c[h, 2 * kp + 1]  # [e,d] weights for odd chunk
            blk = np.empty((128, 2 * D1), dtype=np.float32)
            blk[:, 0::2] = A[:, ::-1]
            blk[:, 1::2] = Bm[:, ::-1]
            c0 = (h * KC + 2 * kp) * D1
            w1p[:, c0:c0 + 2 * D1] = blk
    w1p = np.ascontiguousarray(w1p.astype(_f8))
    # Block-diagonal per-pair [256, 128] -> SwInterleave storage [128, 256]:
    # stored col 2t = sub0 col (127-t), col 2t+1 = sub1 col (127-t), where
    # sub0 = [W2[2j] | 0] over d1 of head 2j, sub1 = [0 | W2[2j+1]].
    w2p = np.zeros((128, NPAIR * 256), dtype=np.float32)
    for j in range(NPAIR):
        sub0 = np.zeros((128, 128), dtype=np.float32)
        sub1 = np.zeros((128, 128), dtype=np.float32)
        sub0[:, 0:64] = W2[2 * j]
        sub1[:, 64:128] = W2[2 * j + 1]
        blk = np.empty((128, 256), dtype=np.float32)
        blk[:, 0::2] = sub0[:, ::-1]
        blk[:, 1::2] = sub1[:, ::-1]
        w2p[:, j * 256:(j + 1) * 256] = blk
    w2p = np.ascontiguousarray(w2p.astype(_f8))
    # Layer 3: per pair a dense [128, 20] block -- rows 0:64 cols 0:10 hold
    # W3[2j], rows 64:128 cols 10:20 hold W3[2j+1] (matching the h2 pair
    # stacking on psum partitions).
    w3p = np.zeros((128, NPAIR * C2), dtype=_bf)
    for j in range(NPAIR):
        w3p[0:64, j * C2:j * C2 + C] = W3[2 * j].astype(_bf)
        w3p[64:128, j * C2 + C:(j + 1) * C2] = W3[2 * j + 1].astype(_bf)
    b1p = np.ascontiguousarray(b1.T)                            # [128, 16]
    b2p = np.ascontiguousarray(b2.reshape(NPAIR, 128).T)        # [128, 8]
    b3_row = b3.reshape(1, OUTC).astype(_bf)
    b3p = np.ascontiguousarray(np.concatenate([b3_row, b3_row], axis=1))

    shared = dict(w1p=w1p, w2p=w2p, w3p=w3p, b1p=b1p, b2p=b2p, b3p=b3p)
    in_maps = []
    for c in range(N_CORES):
        m = dict(shared)
        m["embT"] = np.ascontiguousarray(
            embT[:, c * B_LOC:(c + 1) * B_LOC])
        in_maps.append(m)
    return in_maps


def run(inputs, trace=False):
    """Build, compile and run the SPMD kernel; returns (output, results)."""
    in_maps = prep_inputs(
        inputs["clip_embedding"], inputs["W1"], inputs["b1"],
        inputs["W2"], inputs["b2"], inputs["W3"], inputs["b3"])
    nc = build_program()
    res = run_bass_kernel_spmd(nc, in_maps, list(range(N_CORES)), trace=trace)
    outs = [np.asarray(r["out"], dtype=np.float32) for r in res.results]
    full = np.concatenate(outs, axis=0).reshape(B, H, C)
    return full, res


def kernel(**inputs):
    full, _ = run(inputs)
    return full
